# revision 56
# baseline (speedup 1.0000x reference)
"""Trainium2 Bass kernel for nn_ExpandFormerV15Complete (moe_routing).

Computation (per token t with vocab id v = x[t]):
    h = embed[v]                                  # [64] f32
    A_d = h @ W1[d] + 30*(member[v,d]-1)          # [128] per domain d
    corr = sum_d gelu(A_d) @ (0.1*W2[d])          # one-hot mask folded into
    y = h + corr                                  # the gelu via -30 bias:
                                                  # gelu(a-30) == 0 exactly
Sharding: data-parallel over tokens, 4096 tokens/core on 8 cores.

Gather strategy: the only bulk gather on this HW (dma_gather / ANT ucode)
takes int16 indices and its cost is descriptor-count bound (~9.5ns/idx), so
the 50257-row fused table is packed as row-PAIRS (1024B elements) indexed by
v >> 1, which fits int16 — one descriptor per token instead of two. The
even/odd row is selected on-chip with a per-token parity mask (DVE), and the
gathers are chunked per 512-token group so the expert MLP pipelines against
the gather stream.
"""

import numpy as np
import ml_dtypes

import concourse.bass as bass
import concourse.bacc as bacc
import concourse.tile as tile
import concourse.mybir as mybir
from concourse.bass_utils import run_bass_kernel_spmd

VOCAB = 50257
BASE = 64
NDOM = 8
HID = 128
B, S = 16, 2048
CORR = 0.1
CNEG = 30.0

NCORES = 8
TOK = (B * S) // NCORES          # 4096 tokens per core
P = 128
J = TOK // P                     # 32 token tiles per core
NW = TOK // 16                   # wrapped idx columns
PAIR_ROWS = (VOCAB + 1) // 2     # 25129 row-pairs; idx = v >> 1 fits int16
GROUPS = J // 4                  # 8 groups of 512 tokens

F32 = mybir.dt.float32
BF16 = mybir.dt.bfloat16
I32 = mybir.dt.int32
I16 = mybir.dt.int16


def _install_tile_fix():
    """This walrus build rejects Drain instructions with >1 sync wait.
    Tile's exit barrier attaches one wait per DMA-sem lane to its tail
    drain; split them into a chain of single-wait drains."""
    if getattr(tile.TileContext, "_drain_split_installed", False):
        return

    def _patched(self, tick_clock, wait_clock):
        from concourse.vector_clock import ScopedClock

        drain_inst = self.nc.sync.drain()
        wait_clock.add_sem_waits(
            drain_inst.ins, ScopedClock({None: tick_clock.global_clock})
        )
        si = drain_inst.ins.sync_info
        if si is not None and si.on_wait and len(si.on_wait) > 1:
            waits = list(si.on_wait)
            si.on_wait = waits[:1]
            for w in waits[1:]:
                d2 = self.nc.sync.drain()
                si2 = d2.ins.sync_info
                if si2 is None:
                    d2.ins.sync_info = type(si)(on_wait=[w], on_update=[])
                else:
                    si2.on_wait = list(si2.on_wait) + [w]
        self.nc.all_engine_barrier()
        popped = self.nc._tile_sem_poison_stack.pop()
        assert popped is self._sem_poison
        self.nc.clear_and_free_semaphores(list(self.sems.allocated().values()))
        self.nc.all_engine_barrier()

    tile.TileContext._drain_and_barrier = _patched
    tile.TileContext._drain_split_installed = True


def _build_program():
    _install_tile_fix()
    nc = bacc.Bacc("TRN2", target_bir_lowering=False, debug=False)

    xpw_in = nc.declare_dram_parameter("xpw", [P, NW], I32, isOutput=False)
    par_in = nc.declare_dram_parameter("par", [P, J], F32, isOutput=False)
    t2_in = nc.declare_dram_parameter("t2", [PAIR_ROWS, 256], F32, isOutput=False)
    w1e_in = nc.declare_dram_parameter("w1e", [128, NDOM * HID], BF16, isOutput=False)
    w2s_in = nc.declare_dram_parameter("w2s", [HID, NDOM * BASE], BF16, isOutput=False)
    idn_in = nc.declare_dram_parameter("idn", [128, 128], BF16, isOutput=False)
    y_out = nc.declare_dram_parameter("y", [P, J * BASE], F32, isOutput=True)

    with tile.TileContext(nc) as tc:
        with (
            tc.tile_pool(name="const", bufs=1) as cpool,
            tc.tile_pool(name="gbuf", bufs=1) as gpool,
            tc.tile_pool(name="work", bufs=5) as wpool,
            tc.tile_pool(name="gelu", bufs=12) as glpool,
            tc.tile_pool(name="ps_t", bufs=2, space="PSUM") as ps_t,
            tc.tile_pool(name="ps_a", bufs=3, space="PSUM") as ps_a,
            tc.tile_pool(name="ps_c", bufs=1, space="PSUM") as ps_c,
            tc.tile_pool(name="ps_y", bufs=2, space="PSUM") as ps_y,
        ):
            # ---- constants / inputs to SBUF ----
            xpw = cpool.tile([P, NW], I32)
            nc.sync.dma_start(out=xpw[:, :], in_=xpw_in[:, :])
            par = cpool.tile([P, J], F32)
            nc.sync.dma_start(out=par[:, :], in_=par_in[:, :])
            w1e = cpool.tile([128, NDOM * HID], BF16)
            nc.sync.dma_start(out=w1e[:, :], in_=w1e_in[:, :])
            w2s = cpool.tile([HID, NDOM * BASE], BF16)
            nc.sync.dma_start(out=w2s[:, :], in_=w2s_in[:, :])
            idn = cpool.tile([128, 128], BF16)
            nc.sync.dma_start(out=idn[:, :], in_=idn_in[:, :])

            # pair index (x >> 1) as int16; parity complement mask
            xp16 = cpool.tile([P, NW], I16, tag="idx")
            nc.vector.tensor_copy(out=xp16[:, :], in_=xpw[:, :])
            mpar = cpool.tile([P, J], F32, tag="idx2")
            nc.vector.tensor_scalar(
                mpar[:, :], par[:, :], -1.0, 1.0,
                mybir.AluOpType.mult, mybir.AluOpType.add,
            )


            y_all = gpool.tile([P, J * BASE], F32, tag="yall")

            for gi in range(GROUPS):
                # ---- one chunked pair-gather for this group's 512 tokens ----
                # each 1024B element holds rows (2k, 2k+1) of the fused table
                isl = slice(gi * 32, (gi + 1) * 32)
                jsl = slice(4 * gi, 4 * gi + 4)
                gp = wpool.tile([P, 4 * 256], F32, tag="gp")
                nc.gpsimd.dma_gather(
                    out_ap=gp[:, :].rearrange("p (j e) -> p j e", e=256),
                    in_ap=t2_in[:, :],
                    idxs_ap=xp16[:, isl],
                    num_idxs=512,
                    num_idxs_reg=512,
                    elem_size=256,
                    single_packet=False,
                )
                gp3 = gp[:, :].rearrange("p (j e) -> p j e", e=256)

                # select the parity half: gm = even*(1-par) + odd*par
                parb = par[:, jsl, None].to_broadcast([P, 4, 128])
                mparb = mpar[:, jsl, None].to_broadcast([P, 4, 128])
                gm = wpool.tile([P, 4 * 128], F32, tag="gm")
                gm3 = gm[:, :].rearrange("p (j e) -> p j e", e=128)
                go = wpool.tile([P, 4 * 128], F32, tag="go")
                go3 = go[:, :].rearrange("p (j e) -> p j e", e=128)
                nc.vector.tensor_tensor(
                    out=gm3, in0=gp3[:, :, 0:128], in1=mparb, op=mybir.AluOpType.mult
                )
                nc.vector.tensor_tensor(
                    out=go3, in0=gp3[:, :, 128:256], in1=parb, op=mybir.AluOpType.mult
                )
                nc.vector.tensor_tensor(
                    out=gm[:, :], in0=gm[:, :], in1=go[:, :], op=mybir.AluOpType.add
                )

                # y base: h = gm[:, :, 0:64] (f32, exact)
                ysl = y_all[:, gi * 256 : (gi + 1) * 256].rearrange(
                    "p (j c) -> p j c", c=BASE
                )
                nc.vector.tensor_copy(out=ysl, in_=gm3[:, :, 0:BASE])

                # bf16 copy for the PE transposes
                g16 = wpool.tile([P, 4 * 128], BF16, tag="g16")
                nc.vector.tensor_copy(out=g16[:, :], in_=gm[:, :])

                # transpose the 4 token tiles -> hTm [128 feat, 512 tok] bf16
                pst = ps_t.tile([128, 512], BF16)
                hTm = wpool.tile([128, 512], BF16, tag="hTm")
                for jj in range(4):
                    sl = slice(jj * 128, (jj + 1) * 128)
                    nc.tensor.matmul(
                        pst[:, sl],
                        lhsT=g16[:, sl],
                        rhs=idn[:, :],
                        is_transpose=True,
                        start=True,
                        stop=True,
                    )
                    nc.vector.tensor_copy(out=hTm[:, sl], in_=pst[:, sl])

                # expert MLP, all 8 domains (mask folded in via -30 rows).
                # mm1(d+1) is emitted before mm2(d) so the PE has independent
                # work while ACT computes gelu(d).
                psc = ps_c.tile([BASE, 512], F32)
                Gs = []

                def _mm1(d):
                    psa = ps_a.tile([128, 512], F32)
                    nc.tensor.matmul(
                        psa[:, :],
                        lhsT=w1e[:, d * HID : (d + 1) * HID],
                        rhs=hTm[:, :],
                        start=True,
                        stop=True,
                    )
                    G = glpool.tile([128, 512], BF16, tag="G")
                    nc.scalar.activation(
                        G[:, :], psa[:, :], mybir.ActivationFunctionType.Gelu
                    )
                    Gs.append(G)

                _mm1(0)
                _mm1(1)
                for d in range(NDOM):
                    if d + 2 < NDOM:
                        _mm1(d + 2)
                    nc.tensor.matmul(
                        psc[:, :],
                        lhsT=w2s[:, d * BASE : (d + 1) * BASE],
                        rhs=Gs[d][:, :],
                        start=(d == 0),
                        stop=(d == NDOM - 1),
                    )

                corrT = wpool.tile([BASE, 512], BF16, tag="corrT")
                nc.vector.tensor_copy(out=corrT[:, :], in_=psc[:, :])

                # transpose corr back to token-major and accumulate into y
                psy = ps_y.tile([128, 256], BF16)
                for jj in range(4):
                    nc.tensor.matmul(
                        psy[:, jj * 64 : (jj + 1) * 64],
                        lhsT=corrT[:, jj * 128 : (jj + 1) * 128],
                        rhs=idn[0:BASE, 0:BASE],
                        is_transpose=True,
                        start=True,
                        stop=True,
                    )
                ysl_flat = y_all[:, gi * 256 : (gi + 1) * 256]
                nc.vector.tensor_tensor(
                    out=ysl_flat, in0=ysl_flat, in1=psy[:, :], op=mybir.AluOpType.add
                )
                # stream this group's output out immediately instead of one
                # big write at the end (removes ~3us of serial tail)
                nc.sync.dma_start(
                    out=y_out[:, gi * 256 : (gi + 1) * 256], in_=ysl_flat
                )

    nc.compile()
    return nc


_CACHED_NC = None


def prepare_in_maps(x, embed, W1, W2, member):
    x = np.asarray(x).astype(np.int64).reshape(B * S)
    embed = np.asarray(embed, dtype=np.float32)
    W1 = np.asarray(W1, dtype=np.float32)
    W2 = np.asarray(W2, dtype=np.float32)
    member = np.asarray(member, dtype=np.float32)

    # fused table: cols 0:64 embed, 64:72 = 30*(member-1), rest zero.
    # Packed as row-pairs so the gather index (v >> 1) fits int16.
    fused = np.zeros((2 * PAIR_ROWS, 128), np.float32)
    fused[:VOCAB, :BASE] = embed
    fused[:VOCAB, BASE : BASE + NDOM] = CNEG * (member - 1.0)
    t2 = fused.reshape(PAIR_ROWS, 256)

    w1e = np.zeros((128, NDOM * HID), np.float32)
    for d in range(NDOM):
        w1e[:BASE, d * HID : (d + 1) * HID] = W1[d]
        w1e[BASE + d, d * HID : (d + 1) * HID] = 1.0
    w2s = np.zeros((HID, NDOM * BASE), np.float32)
    for d in range(NDOM):
        w2s[:, d * BASE : (d + 1) * BASE] = W2[d] * CORR
    w1e = w1e.astype(ml_dtypes.bfloat16)
    w2s = w2s.astype(ml_dtypes.bfloat16)
    idn = np.eye(128, dtype=ml_dtypes.bfloat16)

    in_maps = []
    for c in range(NCORES):
        xc = x[c * TOK : (c + 1) * TOK].astype(np.int32)
        xpw = np.tile((xc >> 1).reshape(NW, 16).T, (8, 1)).astype(np.int32)
        # parity in gather-output (token-major) layout: token i at [i%128, i//128]
        par = (xc & 1).astype(np.float32).reshape(J, P).T.copy()
        in_maps.append(
            {"xpw": xpw, "par": par, "t2": t2, "w1e": w1e, "w2s": w2s, "idn": idn}
        )
    return in_maps


def kernel(x, embed, W1, W2, member):
    global _CACHED_NC
    in_maps = prepare_in_maps(x, embed, W1, W2, member)
    if _CACHED_NC is None:
        _CACHED_NC = _build_program()
    nc = _CACHED_NC

    res = run_bass_kernel_spmd(nc, in_maps, core_ids=list(range(NCORES)))

    out = np.empty((B * S, BASE), np.float32)
    for c in range(NCORES):
        yd = res.results[c]["y"].reshape(P, J, BASE)
        out[c * TOK : (c + 1) * TOK] = yd.transpose(1, 0, 2).reshape(TOK, BASE)
    return out.reshape(B, S, BASE)


# revision 57
# speedup vs baseline: 1.0229x; 1.0229x over previous
"""Trainium2 Bass kernel for nn_ExpandFormerV15Complete (moe_routing).

Computation (per token t with vocab id v = x[t]):
    h = embed[v]                                  # [64] f32
    A_d = h @ W1[d] + 30*(member[v,d]-1)          # [128] per domain d
    corr = sum_d gelu(A_d) @ (0.1*W2[d])          # one-hot mask folded into
    y = h + corr                                  # the gelu via -30 bias:
                                                  # gelu(a-30) == 0 exactly
Sharding: data-parallel over tokens, 4096 tokens/core on 8 cores.

Gather strategy: the only bulk gather on this HW (dma_gather / ANT ucode)
takes int16 indices and its cost is descriptor-count bound (~9.5ns/idx), so
the 50257-row fused table is packed as row-PAIRS (1024B elements) indexed by
v >> 1, which fits int16 — one descriptor per token instead of two. The
even/odd row is selected on-chip with a per-token parity mask (DVE), and the
gathers are chunked per 512-token group so the expert MLP pipelines against
the gather stream.
"""

import numpy as np
import ml_dtypes

import concourse.bass as bass
import concourse.bacc as bacc
import concourse.tile as tile
import concourse.mybir as mybir
from concourse.bass_utils import run_bass_kernel_spmd

VOCAB = 50257
BASE = 64
NDOM = 8
HID = 128
B, S = 16, 2048
CORR = 0.1
CNEG = 30.0

NCORES = 8
TOK = (B * S) // NCORES          # 4096 tokens per core
P = 128
J = TOK // P                     # 32 token tiles per core
NW = TOK // 16                   # wrapped idx columns
PAIR_ROWS = (VOCAB + 1) // 2     # 25129 row-pairs; idx = v >> 1 fits int16
GROUPS = J // 4                  # 8 groups of 512 tokens

F32 = mybir.dt.float32
BF16 = mybir.dt.bfloat16
I32 = mybir.dt.int32
I16 = mybir.dt.int16


def _install_tile_fix():
    """This walrus build rejects Drain instructions with >1 sync wait.
    Tile's exit barrier attaches one wait per DMA-sem lane to its tail
    drain; split them into a chain of single-wait drains."""
    if getattr(tile.TileContext, "_drain_split_installed", False):
        return

    def _patched(self, tick_clock, wait_clock):
        from concourse.vector_clock import ScopedClock

        drain_inst = self.nc.sync.drain()
        wait_clock.add_sem_waits(
            drain_inst.ins, ScopedClock({None: tick_clock.global_clock})
        )
        si = drain_inst.ins.sync_info
        if si is not None and si.on_wait and len(si.on_wait) > 1:
            waits = list(si.on_wait)
            si.on_wait = waits[:1]
            for w in waits[1:]:
                d2 = self.nc.sync.drain()
                si2 = d2.ins.sync_info
                if si2 is None:
                    d2.ins.sync_info = type(si)(on_wait=[w], on_update=[])
                else:
                    si2.on_wait = list(si2.on_wait) + [w]
        self.nc.all_engine_barrier()
        popped = self.nc._tile_sem_poison_stack.pop()
        assert popped is self._sem_poison
        self.nc.clear_and_free_semaphores(list(self.sems.allocated().values()))
        self.nc.all_engine_barrier()

    tile.TileContext._drain_and_barrier = _patched
    tile.TileContext._drain_split_installed = True


def _build_program():
    _install_tile_fix()
    nc = bacc.Bacc("TRN2", target_bir_lowering=False, debug=False)

    xpw_in = nc.declare_dram_parameter("xpw", [P, NW], I32, isOutput=False)
    par_in = nc.declare_dram_parameter("par", [P, J], F32, isOutput=False)
    t2_in = nc.declare_dram_parameter("t2", [PAIR_ROWS, 256], F32, isOutput=False)
    w1e_in = nc.declare_dram_parameter("w1e", [128, NDOM * HID], BF16, isOutput=False)
    w2s_in = nc.declare_dram_parameter("w2s", [HID, NDOM * BASE], BF16, isOutput=False)
    idn_in = nc.declare_dram_parameter("idn", [128, 128], BF16, isOutput=False)
    y_out = nc.declare_dram_parameter("y", [P, J * BASE], F32, isOutput=True)

    with tile.TileContext(nc) as tc:
        with (
            tc.tile_pool(name="const", bufs=1) as cpool,
            tc.tile_pool(name="gbuf", bufs=1) as gpool,
            tc.tile_pool(name="work", bufs=5) as wpool,
            tc.tile_pool(name="gelu", bufs=12) as glpool,
            tc.tile_pool(name="ps_t", bufs=1, space="PSUM") as ps_t,
            tc.tile_pool(name="ps_a", bufs=3, space="PSUM") as ps_a,
            tc.tile_pool(name="ps_c", bufs=2, space="PSUM") as ps_c,
            tc.tile_pool(name="ps_y", bufs=2, space="PSUM") as ps_y,
        ):
            # ---- constants / inputs to SBUF ----
            xpw = cpool.tile([P, NW], I32)
            nc.sync.dma_start(out=xpw[:, :], in_=xpw_in[:, :])
            par = cpool.tile([P, J], F32)
            nc.sync.dma_start(out=par[:, :], in_=par_in[:, :])
            w1e = cpool.tile([128, NDOM * HID], BF16)
            nc.sync.dma_start(out=w1e[:, :], in_=w1e_in[:, :])
            w2s = cpool.tile([HID, NDOM * BASE], BF16)
            nc.sync.dma_start(out=w2s[:, :], in_=w2s_in[:, :])
            idn = cpool.tile([128, 128], BF16)
            nc.sync.dma_start(out=idn[:, :], in_=idn_in[:, :])

            # pair index (x >> 1) as int16; parity complement mask
            xp16 = cpool.tile([P, NW], I16, tag="idx")
            nc.vector.tensor_copy(out=xp16[:, :], in_=xpw[:, :])
            mpar = cpool.tile([P, J], F32, tag="idx2")
            nc.vector.tensor_scalar(
                mpar[:, :], par[:, :], -1.0, 1.0,
                mybir.AluOpType.mult, mybir.AluOpType.add,
            )


            y_all = gpool.tile([P, J * BASE], F32, tag="yall")

            for gi in range(GROUPS):
                # ---- one chunked pair-gather for this group's 512 tokens ----
                # each 1024B element holds rows (2k, 2k+1) of the fused table
                isl = slice(gi * 32, (gi + 1) * 32)
                jsl = slice(4 * gi, 4 * gi + 4)
                gp = wpool.tile([P, 4 * 256], F32, tag="gp")
                nc.gpsimd.dma_gather(
                    out_ap=gp[:, :].rearrange("p (j e) -> p j e", e=256),
                    in_ap=t2_in[:, :],
                    idxs_ap=xp16[:, isl],
                    num_idxs=512,
                    num_idxs_reg=512,
                    elem_size=256,
                    single_packet=False,
                )
                gp3 = gp[:, :].rearrange("p (j e) -> p j e", e=256)

                # select the parity half: gm = even*(1-par) + odd*par
                parb = par[:, jsl, None].to_broadcast([P, 4, 128])
                mparb = mpar[:, jsl, None].to_broadcast([P, 4, 128])
                gm = wpool.tile([P, 4 * 128], F32, tag="gm")
                gm3 = gm[:, :].rearrange("p (j e) -> p j e", e=128)
                go = wpool.tile([P, 4 * 128], F32, tag="go")
                go3 = go[:, :].rearrange("p (j e) -> p j e", e=128)
                nc.vector.tensor_tensor(
                    out=gm3, in0=gp3[:, :, 0:128], in1=mparb, op=mybir.AluOpType.mult
                )
                nc.vector.tensor_tensor(
                    out=go3, in0=gp3[:, :, 128:256], in1=parb, op=mybir.AluOpType.mult
                )
                nc.vector.tensor_tensor(
                    out=gm[:, :], in0=gm[:, :], in1=go[:, :], op=mybir.AluOpType.add
                )

                # y base: h = gm[:, :, 0:64] (f32, exact)
                ysl = y_all[:, gi * 256 : (gi + 1) * 256].rearrange(
                    "p (j c) -> p j c", c=BASE
                )
                nc.vector.tensor_copy(out=ysl, in_=gm3[:, :, 0:BASE])

                # bf16 copy for the PE transposes
                g16 = wpool.tile([P, 4 * 128], BF16, tag="g16")
                nc.vector.tensor_copy(out=g16[:, :], in_=gm[:, :])

                # transpose the 4 token tiles -> hTm [128 feat, 512 tok] bf16
                pst = ps_t.tile([128, 512], BF16)
                hTm = wpool.tile([128, 512], BF16, tag="hTm")
                for jj in range(4):
                    sl = slice(jj * 128, (jj + 1) * 128)
                    nc.tensor.matmul(
                        pst[:, sl],
                        lhsT=g16[:, sl],
                        rhs=idn[:, :],
                        is_transpose=True,
                        start=True,
                        stop=True,
                    )
                    nc.vector.tensor_copy(out=hTm[:, sl], in_=pst[:, sl])

                # expert MLP, all 8 domains (mask folded in via -30 rows).
                # mm1(d+1) is emitted before mm2(d) so the PE has independent
                # work while ACT computes gelu(d).
                psc = ps_c.tile([BASE, 512], F32)
                Gs = []

                def _mm1(d):
                    psa = ps_a.tile([128, 512], F32)
                    nc.tensor.matmul(
                        psa[:, :],
                        lhsT=w1e[:, d * HID : (d + 1) * HID],
                        rhs=hTm[:, :],
                        start=True,
                        stop=True,
                    )
                    G = glpool.tile([128, 512], BF16, tag="G")
                    nc.scalar.activation(
                        G[:, :], psa[:, :], mybir.ActivationFunctionType.Gelu
                    )
                    Gs.append(G)

                _mm1(0)
                _mm1(1)
                for d in range(NDOM):
                    if d + 2 < NDOM:
                        _mm1(d + 2)
                    nc.tensor.matmul(
                        psc[:, :],
                        lhsT=w2s[:, d * BASE : (d + 1) * BASE],
                        rhs=Gs[d][:, :],
                        start=(d == 0),
                        stop=(d == NDOM - 1),
                    )

                corrT = wpool.tile([BASE, 512], BF16, tag="corrT")
                nc.vector.tensor_copy(out=corrT[:, :], in_=psc[:, :])

                # transpose corr back to token-major and accumulate into y
                psy = ps_y.tile([128, 256], BF16)
                for jj in range(4):
                    nc.tensor.matmul(
                        psy[:, jj * 64 : (jj + 1) * 64],
                        lhsT=corrT[:, jj * 128 : (jj + 1) * 128],
                        rhs=idn[0:BASE, 0:BASE],
                        is_transpose=True,
                        start=True,
                        stop=True,
                    )
                ysl_flat = y_all[:, gi * 256 : (gi + 1) * 256]
                nc.vector.tensor_tensor(
                    out=ysl_flat, in0=ysl_flat, in1=psy[:, :], op=mybir.AluOpType.add
                )
                # stream this group's output out immediately instead of one
                # big write at the end (removes ~3us of serial tail)
                nc.sync.dma_start(
                    out=y_out[:, gi * 256 : (gi + 1) * 256], in_=ysl_flat
                )

    nc.compile()
    return nc


_CACHED_NC = None


def prepare_in_maps(x, embed, W1, W2, member):
    x = np.asarray(x).astype(np.int64).reshape(B * S)
    embed = np.asarray(embed, dtype=np.float32)
    W1 = np.asarray(W1, dtype=np.float32)
    W2 = np.asarray(W2, dtype=np.float32)
    member = np.asarray(member, dtype=np.float32)

    # fused table: cols 0:64 embed, 64:72 = 30*(member-1), rest zero.
    # Packed as row-pairs so the gather index (v >> 1) fits int16.
    fused = np.zeros((2 * PAIR_ROWS, 128), np.float32)
    fused[:VOCAB, :BASE] = embed
    fused[:VOCAB, BASE : BASE + NDOM] = CNEG * (member - 1.0)
    t2 = fused.reshape(PAIR_ROWS, 256)

    w1e = np.zeros((128, NDOM * HID), np.float32)
    for d in range(NDOM):
        w1e[:BASE, d * HID : (d + 1) * HID] = W1[d]
        w1e[BASE + d, d * HID : (d + 1) * HID] = 1.0
    w2s = np.zeros((HID, NDOM * BASE), np.float32)
    for d in range(NDOM):
        w2s[:, d * BASE : (d + 1) * BASE] = W2[d] * CORR
    w1e = w1e.astype(ml_dtypes.bfloat16)
    w2s = w2s.astype(ml_dtypes.bfloat16)
    idn = np.eye(128, dtype=ml_dtypes.bfloat16)

    in_maps = []
    for c in range(NCORES):
        xc = x[c * TOK : (c + 1) * TOK].astype(np.int32)
        xpw = np.tile((xc >> 1).reshape(NW, 16).T, (8, 1)).astype(np.int32)
        # parity in gather-output (token-major) layout: token i at [i%128, i//128]
        par = (xc & 1).astype(np.float32).reshape(J, P).T.copy()
        in_maps.append(
            {"xpw": xpw, "par": par, "t2": t2, "w1e": w1e, "w2s": w2s, "idn": idn}
        )
    return in_maps


def kernel(x, embed, W1, W2, member):
    global _CACHED_NC
    in_maps = prepare_in_maps(x, embed, W1, W2, member)
    if _CACHED_NC is None:
        _CACHED_NC = _build_program()
    nc = _CACHED_NC

    res = run_bass_kernel_spmd(nc, in_maps, core_ids=list(range(NCORES)))

    out = np.empty((B * S, BASE), np.float32)
    for c in range(NCORES):
        yd = res.results[c]["y"].reshape(P, J, BASE)
        out[c * TOK : (c + 1) * TOK] = yd.transpose(1, 0, 2).reshape(TOK, BASE)
    return out.reshape(B, S, BASE)
